# revision 5
# baseline (speedup 1.0000x reference)
"""Trainium2 Bass kernel for multi-head attention graph scatter.

Computes, for each of 8 heads h (one NeuronCore per head):
    q_h = query @ w_q[:, h*32:(h+1)*32]          # [3000, 32]
    k_h = key_emb @ w_k[:, h*32:(h+1)*32]        # [4096, 32]
    attn_h = softmax(q_h @ k_h.T / sqrt(32))     # [3000, 4096]
    graphs[h, qt, :] = attn_h                    # [4096, 4096], rest zeros

kernel(**inputs) takes the full (unsharded) numpy inputs and returns the
full [8, 4096, 4096] float32 output.
"""

import math
import sys

import numpy as np

if "/opt/trn_rl_repo" not in sys.path:
    sys.path.insert(0, "/opt/trn_rl_repo")

N_HEAD = 8
D_K = 32
CONCEPT_NUM = 4096
MASK_NUM = 3000
INPUT_DIM = 256

P = 128  # SBUF partitions
NBLK = 512  # matmul moving-dim tile (one PSUM bank of f32)

_BUILD_CACHE = {}


def _build_module():
    """Build the per-core Bass module (identical on all 8 cores; inputs differ)."""
    import concourse.bacc as bacc
    import concourse.mybir as mybir
    import concourse.tile as tile
    from concourse.masks import make_identity

    f32 = mybir.dt.float32
    f32r = mybir.dt.float32r
    SCALE = 1.0 / math.sqrt(D_K)

    nc = bacc.Bacc("TRN2", target_bir_lowering=False, debug=False, num_devices=N_HEAD)

    query = nc.dram_tensor("query", [MASK_NUM, INPUT_DIM], f32, kind="ExternalInput")
    key_emb = nc.dram_tensor("key_emb", [CONCEPT_NUM, INPUT_DIM], f32, kind="ExternalInput")
    w_qh = nc.dram_tensor("w_qh", [INPUT_DIM, D_K], f32, kind="ExternalInput")
    w_kh = nc.dram_tensor("w_kh", [INPUT_DIM, D_K], f32, kind="ExternalInput")
    graphs = nc.dram_tensor("graphs", [CONCEPT_NUM, CONCEPT_NUM], f32, kind="ExternalOutput")

    # mask-dim tiling: 3000 = 23*128 + 56
    m_tiles = [P] * (MASK_NUM // P) + ([MASK_NUM % P] if MASK_NUM % P else [])
    n_mt = len(m_tiles)
    # concept-dim tiling for keyT / kT: 4096 = 8 * 512
    n_kc = CONCEPT_NUM // NBLK
    # mask-dim chunks for queryT / qT: 3000 = 5*512 + 440
    q_chunks = [NBLK] * (MASK_NUM // NBLK) + ([MASK_NUM % NBLK] if MASK_NUM % NBLK else [])
    n_qc = len(q_chunks)

    with tile.TileContext(nc) as tc:
        with (
            tc.tile_pool(name="const", bufs=1) as const_pool,
            tc.tile_pool(name="loads", bufs=3) as loads,
            tc.tile_pool(name="trans", bufs=1) as trans_pool,
            tc.tile_pool(name="proj", bufs=1) as proj_pool,
            tc.tile_pool(name="stats", bufs=4) as stats,
            tc.tile_pool(name="expp", bufs=3) as expp,
            tc.tile_pool(name="tpsum", bufs=2, space="PSUM") as tpsum,
            tc.tile_pool(name="ppsum", bufs=2, space="PSUM") as ppsum,
            tc.tile_pool(name="mpsum", bufs=2, space="PSUM") as mpsum,
        ):
            identity = const_pool.tile([P, P], f32)
            make_identity(nc, identity)

            # w slices in lhsT layout: [128, 2, 32] where [p, a, j] = w[a*128+p, j]
            wq_sb = const_pool.tile([P, 2, D_K], f32)
            nc.sync.dma_start(wq_sb[:], w_qh.ap().rearrange("(a p) j -> p a j", p=P))
            wk_sb = const_pool.tile([P, 2, D_K], f32)
            nc.sync.dma_start(wk_sb[:], w_kh.ap().rearrange("(a p) j -> p a j", p=P))

            # ---- key side: key_emb [4096, 256] -> key_embT (2 x 8 tiles of [128, 512])
            # keyT[a][j][p, c'] = key_emb[j*512 + c', a*128 + p]
            keyT = [
                [trans_pool.tile([P, NBLK], f32, tag=f"keyT{a}_{j}", name=f"keyT{a}_{j}") for j in range(n_kc)]
                for a in range(2)
            ]
            key_r = key_emb.ap().rearrange("(t p) d -> p t d", p=P)  # [128, 32, 256]
            n_kt = CONCEPT_NUM // P  # 32 row-tiles
            for g in range(4):  # load in 4 chunks of 8 row-tiles (1 MB each)
                ktile = loads.tile([P, 8, INPUT_DIM], f32, tag="kload")
                nc.sync.dma_start(ktile[:], key_r[:, g * 8 : (g + 1) * 8, :])
                for t in range(8):
                    ct = g * 8 + t
                    for a in range(2):
                        tp = tpsum.tile([P, P], f32, tag="tp")
                        nc.tensor.transpose(tp[:], ktile[:, t, a * P : (a + 1) * P], identity[:])
                        dst = keyT[a][ct // 4]
                        col = (ct % 4) * P
                        if ct % 2 == 0:
                            nc.vector.tensor_copy(dst[:, col : col + P], tp[:])
                        else:
                            nc.scalar.copy(dst[:, col : col + P], tp[:])

            # ---- query side: query [3000, 256] -> queryT (2 x 6 tiles of [128, <=512])
            queryT = [
                [trans_pool.tile([P, q_chunks[j]], f32, tag=f"queryT{a}_{j}", name=f"queryT{a}_{j}") for j in range(n_qc)]
                for a in range(2)
            ]
            n_qt_full = MASK_NUM // P  # 23 full row-tiles
            qfull = query.ap()[: n_qt_full * P, :].rearrange("(t p) d -> p t d", p=P)
            for g in range(6):  # 6 groups of <=4 row-tiles
                t0, t1 = g * 4, min((g + 1) * 4, n_qt_full)
                qtile = loads.tile([P, 4, INPUT_DIM], f32, tag="qload")
                nc.sync.dma_start(qtile[:, : t1 - t0, :], qfull[:, t0:t1, :])
                for t in range(t1 - t0):
                    mt = t0 + t
                    for a in range(2):
                        tp = tpsum.tile([P, P], f32, tag="tp")
                        nc.tensor.transpose(tp[:], qtile[:, t, a * P : (a + 1) * P], identity[:])
                        dst = queryT[a][mt // 4]
                        col = (mt % 4) * P
                        if mt % 2 == 0:
                            nc.vector.tensor_copy(dst[:, col : col + P], tp[:])
                        else:
                            nc.scalar.copy(dst[:, col : col + P], tp[:])
            # last partial row-tile (56 rows)
            mrem = MASK_NUM - n_qt_full * P
            qtile = loads.tile([P, 4, INPUT_DIM], f32, tag="qload")
            nc.sync.dma_start(
                qtile[:mrem, 0, :], query.ap()[n_qt_full * P :, :]
            )
            for a in range(2):
                tp = tpsum.tile([P, P], f32, tag="tp")
                nc.tensor.transpose(
                    tp[:, :mrem], qtile[:mrem, 0, a * P : (a + 1) * P], identity[:mrem, :mrem]
                )
                dst = queryT[a][n_qc - 1]
                col = q_chunks[n_qc - 1] - mrem
                nc.vector.tensor_copy(dst[:, col : col + mrem], tp[:, :mrem])

            # ---- projections: kT[j] [32, 512] = w_kh.T @ key_embT chunk
            kT = [proj_pool.tile([D_K, NBLK], f32, tag=f"kT_{j}", name=f"kT_{j}") for j in range(n_kc)]
            for j in range(n_kc):
                ps = ppsum.tile([D_K, NBLK], f32, tag="pps")
                nc.tensor.matmul(ps[:], wk_sb[:, 0, :], keyT[0][j][:], start=True, stop=False)
                nc.tensor.matmul(ps[:], wk_sb[:, 1, :], keyT[1][j][:], start=False, stop=True)
                nc.vector.tensor_copy(kT[j][:].bitcast(f32r), ps[:])

            qT = [proj_pool.tile([D_K, q_chunks[j]], f32, tag=f"qT_{j}", name=f"qT_{j}") for j in range(n_qc)]
            for j in range(n_qc):
                w = q_chunks[j]
                ps = ppsum.tile([D_K, NBLK], f32, tag="pps")
                nc.tensor.matmul(ps[:, :w], wq_sb[:, 0, :], queryT[0][j][:], start=True, stop=False)
                nc.tensor.matmul(ps[:, :w], wq_sb[:, 1, :], queryT[1][j][:], start=False, stop=True)
                nc.vector.tensor_copy(qT[j][:].bitcast(f32r), ps[:, :w])

            # ---- main loop: scores -> exp (+row sums) -> normalize -> DMA out
            for i in range(n_mt):
                mt = m_tiles[i]
                # lhsT = qT slice [32, mt]: row-tile i lives in chunk i//4
                cj = i // 4
                c0 = i * P - cj * NBLK
                lhsT = qT[cj][:, c0 : c0 + mt].bitcast(f32r)

                exp_t = expp.tile([P, CONCEPT_NUM], f32, tag="exp")
                sums = stats.tile([P, 4], f32, tag="sums")
                tot = stats.tile([P, 1], f32, tag="tot")
                rec = stats.tile([P, 1], f32, tag="rec")

                for h4 in range(4):
                    ps = mpsum.tile([P, 2 * NBLK], f32, tag="mps")
                    for j2 in range(2):
                        j = h4 * 2 + j2
                        nc.tensor.matmul(
                            ps[:mt, j2 * NBLK : (j2 + 1) * NBLK],
                            lhsT,
                            kT[j][:].bitcast(f32r),
                            start=True,
                            stop=True,
                        )
                    nc.scalar.activation(
                        exp_t[:mt, h4 * 2 * NBLK : (h4 + 1) * 2 * NBLK],
                        ps[:mt, :],
                        mybir.ActivationFunctionType.Exp,
                        scale=SCALE,
                        accum_out=sums[:mt, h4 : h4 + 1],
                    )
                nc.vector.tensor_reduce(
                    tot[:mt], sums[:mt, :], axis=mybir.AxisListType.X, op=mybir.AluOpType.add
                )
                nc.vector.reciprocal(rec[:mt], tot[:mt])
                nc.vector.tensor_scalar_mul(exp_t[:mt, :], exp_t[:mt, :], rec[:mt])
                nc.sync.dma_start(graphs.ap()[i * P : i * P + mt, :], exp_t[:mt, :])

    nc.compile()
    return nc


def _get_module():
    if "nc" not in _BUILD_CACHE:
        _BUILD_CACHE["nc"] = _build_module()
    return _BUILD_CACHE["nc"]


def kernel(qt, query, key_emb, w_q, w_k):
    from concourse.bass_utils import run_bass_kernel_spmd

    qt = np.asarray(qt)
    query = np.ascontiguousarray(np.asarray(query, dtype=np.float32))
    key_emb = np.ascontiguousarray(np.asarray(key_emb, dtype=np.float32))
    w_q = np.asarray(w_q, dtype=np.float32)
    w_k = np.asarray(w_k, dtype=np.float32)

    nc = _get_module()
    in_maps = []
    for h in range(N_HEAD):
        in_maps.append(
            {
                "query": query,
                "key_emb": key_emb,
                "w_qh": np.ascontiguousarray(w_q[:, h * D_K : (h + 1) * D_K]),
                "w_kh": np.ascontiguousarray(w_k[:, h * D_K : (h + 1) * D_K]),
            }
        )
    res = run_bass_kernel_spmd(nc, in_maps, core_ids=list(range(N_HEAD)))
    out = np.stack([res.results[h]["graphs"] for h in range(N_HEAD)], axis=0)

    # Device assumes qt == arange(3000) (rows land at graph rows 0..2999,
    # remaining rows stay zero). Remap on host for any other qt.
    if not np.array_equal(qt, np.arange(MASK_NUM)):
        full = np.zeros((N_HEAD, CONCEPT_NUM, CONCEPT_NUM), dtype=np.float32)
        full[:, qt.astype(np.int64), :] = out[:, :MASK_NUM, :]
        out = full
    return out
